# revision 13
# baseline (speedup 1.0000x reference)
"""AcceptRejectPooling2D on 8 Trainium2 NeuronCores.

Reference semantics (per 2x2 window, stride 2, NHWC):
    r  = relu(x)
    s  = sum(r); ss = sum(r*r)
    out = ss / s   if s > 0 else 0

Sharding: pure data parallel over batch (64 -> 8 per core). Each core
processes x_local [8, 64, 64, 256] -> y_local [8, 32, 32, 256].

Pipeline (v4): partitions carry (bb, h) = 2 batches x 64 input rows.
  ACT:  R = relu(x) -> bf16
  DVE:  w-pair adds in bf16 (2x packed mode); Q = R*R on 2/3 of chunks
  PE :  h-pair reduction as matmul with a 0/1 halving matrix
        W[k, m] = 1 iff m == k//2; two groups pack into one PSUM tile
        ([0:64) from group A via W_A, [64:128) from group B via W_B);
        a third K=1 matmul accumulates eps into s so 0-windows give 0/eps
  DVE:  t = 1/s (custom recip), out = ss * t, both reading PSUM f32
"""

import sys

if "/opt/trn_rl_repo" not in sys.path:
    sys.path.insert(0, "/opt/trn_rl_repo")

import numpy as np

_B, _H, _W, _C = 8, 64, 64, 256  # per-core shard
_HO, _WO = _H // 2, _W // 2
_NP = 128                         # SBUF partitions
_F = 4096                         # floats per row chunk (16 w * 256 c)
_FH = _F // 2                     # w-pair reduced width
_PC = 512                         # psum piece width (f32, 1 bank)
_NG = 4                           # groups of (bb=2, h=64) partition rows
_EPS = 1e-30

_CACHE = {}


def _pin_act_table(bacc, mybir):
    """Route every activation to natural_log_exp_and_others so the kernel
    needs exactly one ACT function-table load. The compiler's greedy set
    choice otherwise alternates sets (~2.7us reload each)."""
    if getattr(bacc, "_arp_act_pin", False):
        return
    AF = mybir.ActivationFunctionType
    pin = {AF.Relu, AF.Square, AF.Ln, AF.Exp}
    orig = bacc.get_activation_tables

    def pinned(arch):
        tabs = orig(arch)
        keep = {f for f in pin if f in tabs.get("natural_log_exp_and_others", set())}
        return {
            name: (fns if name == "natural_log_exp_and_others" else fns - keep)
            for name, fns in tabs.items()
        }

    bacc.get_activation_tables = pinned
    bacc._arp_act_pin = True


def _build_nc():
    import concourse.bacc as bacc
    import concourse.tile as tile
    from concourse import mybir

    _pin_act_table(bacc, mybir)
    nc = bacc.Bacc("TRN2", target_bir_lowering=False, debug=False, num_devices=8)
    f32 = mybir.dt.float32
    bf16 = mybir.dt.bfloat16
    i32 = mybir.dt.int32
    x = nc.dram_tensor("x", [_B, _H, _W, _C], f32, kind="ExternalInput")
    y = nc.dram_tensor("y", [_B, _HO, _WO, _C], f32, kind="ExternalOutput")

    # [4, 128, 16384]: xg[bg, (bb, h), (w, c)]
    xg = x.ap().rearrange("(bg bb) h w c -> bg (bb h) (w c)", bb=2)
    # [2, 128, 8192]: yo[pair, (half, bb, ho), (wo, c)]
    yo = y.ap().rearrange("(pr hf bb) ho w c -> pr (hf bb ho) (w c)", pr=2, hf=2)

    relu = mybir.ActivationFunctionType.Relu
    square = mybir.ActivationFunctionType.Square
    is_ge = mybir.AluOpType.is_ge
    is_lt = mybir.AluOpType.is_lt

    with tile.TileContext(nc) as tc:
        with (
            tc.tile_pool(name="io", bufs=4) as io,
            tc.tile_pool(name="rq", bufs=4) as rq,
            tc.tile_pool(name="tmp", bufs=3) as tmp,
            tc.tile_pool(name="ot", bufs=3) as ot,
            tc.tile_pool(name="wt", bufs=1) as wt,
            tc.psum_pool(name="ps", bufs=3) as ps,
        ):
            # --- one-time: build the two halving matrices in SBUF ---
            # W_A[k, j] = 1 iff j == k//2       (cols 64.. are zero)
            # W_B[k, j] = 1 iff j == 64 + k//2  (cols ..64 are zero)
            WA = wt.tile([_NP, _NP], bf16, tag="WA")
            WB = wt.tile([_NP, _NP], bf16, tag="WB")
            d = wt.tile([_NP, _NP], i32, tag="d")
            ge = wt.tile([_NP, _NP], i32, tag="ge")
            lt = wt.tile([_NP, _NP], i32, tag="lt")
            wi = wt.tile([_NP, _NP], i32, tag="wi")
            for W, base in ((WA, 0), (WB, 128)):
                # d[p, j] = base + p - 2j; W = (d >= 0) & (d < 2)
                nc.gpsimd.iota(d[:], [[-2, _NP]], base=base, channel_multiplier=1)
                nc.vector.tensor_scalar(ge[:], d[:], 0, None, op0=is_ge)
                nc.vector.tensor_scalar(lt[:], d[:], 2, None, op0=is_lt)
                nc.vector.tensor_mul(wi[:], ge[:], lt[:])
                nc.vector.tensor_copy(W[:], wi[:])

            # warm the ACT table load + DVE recip custom-op path
            warm0 = wt.tile([_NP, 8], f32, tag="warm0")
            warm1 = wt.tile([_NP, 8], f32, tag="warm1")
            warmb = wt.tile([_NP, 8], bf16, tag="warmb")
            nc.vector.memset(warm0[:], 1.0)
            nc.scalar.activation(warmb[:], warm0[:], relu)
            nc.scalar.activation(warmb[:], warmb[:], square)
            nc.vector.reciprocal_approx_fast(warm1[:], warm0[:])

            # eps injectors: ones[1,128].T @ epsrow[1,N] accumulates eps
            # into every element of an s psum tile (runs on the idle PE)
            WE = wt.tile([1, _NP], bf16, tag="WE")
            epsr = wt.tile([1, _PC], bf16, tag="epsr")
            nc.vector.memset(WE[:], 1.0)
            nc.vector.memset(epsr[:], _EPS)

            def front(bg, c0, act_square):
                """Load + relu + square + w-pair adds for one group chunk.
                Returns (sw, ssw) bf16 [128, F/2] tiles."""
                EO = io.tile([_NP, _F], f32, tag="EO")
                nc.sync.dma_start(EO[:], xg[bg, :, c0:c0 + _F])
                R = rq.tile([_NP, _F], bf16, tag="RQ")
                Q = rq.tile([_NP, _F], bf16, tag="RQ")
                sw = tmp.tile([_NP, _FH], bf16, tag="sw")
                ssw = tmp.tile([_NP, _FH], bf16, tag="ssw")

                def wpair(t_):
                    v = t_[:].rearrange("p (w par c) -> p w par c", par=2, c=_C)
                    return v[:, :, 0, :], v[:, :, 1, :]

                def whalf(t_):
                    return t_[:].rearrange("p (w c) -> p w c", c=_C)

                nc.scalar.activation(R[:], EO[:], relu)
                Re, Ro = wpair(R)
                nc.vector.tensor_add(whalf(sw), Re, Ro)
                if act_square:
                    nc.scalar.activation(Q[:], R[:], square)
                else:
                    nc.vector.tensor_mul(Q[:], R[:], R[:])
                Qe, Qo = wpair(Q)
                nc.vector.tensor_add(whalf(ssw), Qe, Qo)
                return sw, ssw

            # schedule: pairs of groups; within a pair, chunk columns
            sq_idx = 0
            for pair in range(2):
                bgA, bgB = 2 * pair, 2 * pair + 1
                for c0 in range(0, _W * _C, _F):
                    swA, sswA = front(bgA, c0, sq_idx % 3 == 2)
                    sq_idx += 1
                    swB, sswB = front(bgB, c0, sq_idx % 3 == 2)
                    sq_idx += 1
                    o = ot.tile([_NP, _FH], f32, tag="o")
                    for pc in range(0, _FH, _PC):
                        s_ps = ps.tile([_NP, _PC], f32, tag="s")
                        q_ps = ps.tile([_NP, _PC], f32, tag="q")
                        nc.tensor.matmul(
                            s_ps[:], WA[:], swA[:, pc:pc + _PC],
                            start=True, stop=False,
                        )
                        nc.tensor.matmul(
                            s_ps[:], WB[:], swB[:, pc:pc + _PC],
                            start=False, stop=False,
                        )
                        nc.tensor.matmul(
                            s_ps[:], WE[:], epsr[:],
                            start=False, stop=True,
                        )
                        nc.tensor.matmul(
                            q_ps[:], WA[:], sswA[:, pc:pc + _PC],
                            start=True, stop=False,
                        )
                        nc.tensor.matmul(
                            q_ps[:], WB[:], sswB[:, pc:pc + _PC],
                            start=False, stop=True,
                        )
                        t = tmp.tile([_NP, _PC], f32, tag="t")
                        nc.vector.reciprocal_approx_fast(t[:], s_ps[:])
                        nc.vector.tensor_mul(o[:, pc:pc + _PC], q_ps[:], t[:])
                    nc.sync.dma_start(
                        yo[pair, :, c0 // 2:c0 // 2 + _FH], o[:]
                    )

    nc.compile()
    return nc


def _get_nc():
    if "nc" not in _CACHE:
        _CACHE["nc"] = _build_nc()
    return _CACHE["nc"]


def kernel(x: np.ndarray) -> np.ndarray:
    from concourse.bass_utils import run_bass_kernel_spmd

    nc = _get_nc()
    x = np.ascontiguousarray(np.asarray(x, dtype=np.float32))
    shards = np.split(x, 8, axis=0)
    in_maps = [{"x": s} for s in shards]
    res = run_bass_kernel_spmd(nc, in_maps, list(range(8)))
    return np.concatenate([res.results[i]["y"] for i in range(8)], axis=0)


# revision 14
# speedup vs baseline: 1.0157x; 1.0157x over previous
"""AcceptRejectPooling2D on 8 Trainium2 NeuronCores.

Reference semantics (per 2x2 window, stride 2, NHWC):
    r  = relu(x)
    s  = sum(r); ss = sum(r*r)
    out = ss / s   if s > 0 else 0

Sharding: pure data parallel over batch (64 -> 8 per core). Each core
processes x_local [8, 64, 64, 256] -> y_local [8, 32, 32, 256].

Pipeline (v4): partitions carry (bb, h) = 2 batches x 64 input rows.
  ACT:  R = relu(x) -> bf16
  DVE:  w-pair adds in bf16 (2x packed mode); Q = R*R on 2/3 of chunks
  PE :  h-pair reduction as matmul with a 0/1 halving matrix
        W[k, m] = 1 iff m == k//2; two groups pack into one PSUM tile
        ([0:64) from group A via W_A, [64:128) from group B via W_B);
        a third K=1 matmul accumulates eps into s so 0-windows give 0/eps
  DVE:  t = 1/s (custom recip), out = ss * t, both reading PSUM f32
"""

import sys

if "/opt/trn_rl_repo" not in sys.path:
    sys.path.insert(0, "/opt/trn_rl_repo")

import numpy as np

_B, _H, _W, _C = 8, 64, 64, 256  # per-core shard
_HO, _WO = _H // 2, _W // 2
_NP = 128                         # SBUF partitions
_F = 4096                         # floats per row chunk (16 w * 256 c)
_FH = _F // 2                     # w-pair reduced width
_PC = 512                         # psum piece width (f32, 1 bank)
_NG = 4                           # groups of (bb=2, h=64) partition rows
_EPS = 1e-30

_CACHE = {}


def _pin_act_table(bacc, mybir):
    """Route every activation to natural_log_exp_and_others so the kernel
    needs exactly one ACT function-table load. The compiler's greedy set
    choice otherwise alternates sets (~2.7us reload each)."""
    if getattr(bacc, "_arp_act_pin", False):
        return
    AF = mybir.ActivationFunctionType
    pin = {AF.Relu, AF.Square, AF.Ln, AF.Exp}
    orig = bacc.get_activation_tables

    def pinned(arch):
        tabs = orig(arch)
        keep = {f for f in pin if f in tabs.get("natural_log_exp_and_others", set())}
        return {
            name: (fns if name == "natural_log_exp_and_others" else fns - keep)
            for name, fns in tabs.items()
        }

    bacc.get_activation_tables = pinned
    bacc._arp_act_pin = True


def _build_nc():
    import concourse.bacc as bacc
    import concourse.tile as tile
    from concourse import mybir

    _pin_act_table(bacc, mybir)
    nc = bacc.Bacc("TRN2", target_bir_lowering=False, debug=False, num_devices=8)
    f32 = mybir.dt.float32
    bf16 = mybir.dt.bfloat16
    i32 = mybir.dt.int32
    x = nc.dram_tensor("x", [_B, _H, _W, _C], f32, kind="ExternalInput")
    y = nc.dram_tensor("y", [_B, _HO, _WO, _C], f32, kind="ExternalOutput")

    # [4, 128, 16384]: xg[bg, (bb, h), (w, c)]
    xg = x.ap().rearrange("(bg bb) h w c -> bg (bb h) (w c)", bb=2)
    # [2, 128, 8192]: yo[pair, (half, bb, ho), (wo, c)]
    yo = y.ap().rearrange("(pr hf bb) ho w c -> pr (hf bb ho) (w c)", pr=2, hf=2)

    relu = mybir.ActivationFunctionType.Relu
    square = mybir.ActivationFunctionType.Square
    is_ge = mybir.AluOpType.is_ge
    is_lt = mybir.AluOpType.is_lt

    with tile.TileContext(nc) as tc:
        with (
            tc.tile_pool(name="io", bufs=4) as io,
            tc.tile_pool(name="rq", bufs=8) as rq,
            tc.tile_pool(name="tmp", bufs=4) as tmp,
            tc.tile_pool(name="ot", bufs=3) as ot,
            tc.tile_pool(name="wt", bufs=1) as wt,
            tc.psum_pool(name="ps", bufs=3) as ps,
        ):
            # --- one-time: build the two halving matrices in SBUF ---
            # W_A[k, j] = 1 iff j == k//2       (cols 64.. are zero)
            # W_B[k, j] = 1 iff j == 64 + k//2  (cols ..64 are zero)
            WA = wt.tile([_NP, _NP], bf16, tag="WA")
            WB = wt.tile([_NP, _NP], bf16, tag="WB")
            d = wt.tile([_NP, _NP], i32, tag="d")
            ge = wt.tile([_NP, _NP], i32, tag="ge")
            lt = wt.tile([_NP, _NP], i32, tag="lt")
            wi = wt.tile([_NP, _NP], i32, tag="wi")
            for W, base in ((WA, 0), (WB, 128)):
                # d[p, j] = base + p - 2j; W = (d >= 0) & (d < 2)
                nc.gpsimd.iota(d[:], [[-2, _NP]], base=base, channel_multiplier=1)
                nc.vector.tensor_scalar(ge[:], d[:], 0, None, op0=is_ge)
                nc.vector.tensor_scalar(lt[:], d[:], 2, None, op0=is_lt)
                nc.vector.tensor_mul(wi[:], ge[:], lt[:])
                nc.vector.tensor_copy(W[:], wi[:])

            # warm the ACT table load + DVE recip custom-op path
            warm0 = wt.tile([_NP, 8], f32, tag="warm0")
            warm1 = wt.tile([_NP, 8], f32, tag="warm1")
            warmb = wt.tile([_NP, 8], bf16, tag="warmb")
            nc.vector.memset(warm0[:], 1.0)
            nc.scalar.activation(warmb[:], warm0[:], relu)
            nc.scalar.activation(warmb[:], warmb[:], square)
            nc.vector.reciprocal_approx_fast(warm1[:], warm0[:])

            # eps injectors: ones[1,128].T @ epsrow[1,N] accumulates eps
            # into every element of an s psum tile (runs on the idle PE)
            WE = wt.tile([1, _NP], bf16, tag="WE")
            epsr = wt.tile([1, _PC], bf16, tag="epsr")
            nc.vector.memset(WE[:], 1.0)
            nc.vector.memset(epsr[:], _EPS)

            def front(bg, c0, act_square):
                """Load + relu + square + w-pair adds for one group chunk.
                Returns (sw, ssw) bf16 [128, F/2] tiles."""
                EO = io.tile([_NP, _F], f32, tag="EO")
                nc.sync.dma_start(EO[:], xg[bg, :, c0:c0 + _F])
                R = rq.tile([_NP, _F], bf16, tag="RQ")
                Q = rq.tile([_NP, _F], bf16, tag="RQ")
                sw = tmp.tile([_NP, _FH], bf16, tag="sw")
                ssw = tmp.tile([_NP, _FH], bf16, tag="ssw")

                def wpair(t_):
                    v = t_[:].rearrange("p (w par c) -> p w par c", par=2, c=_C)
                    return v[:, :, 0, :], v[:, :, 1, :]

                def whalf(t_):
                    return t_[:].rearrange("p (w c) -> p w c", c=_C)

                nc.scalar.activation(R[:], EO[:], relu)
                Re, Ro = wpair(R)
                nc.vector.tensor_add(whalf(sw), Re, Ro)
                if act_square:
                    nc.scalar.activation(Q[:], R[:], square)
                else:
                    nc.vector.tensor_mul(Q[:], R[:], R[:])
                Qe, Qo = wpair(Q)
                nc.vector.tensor_add(whalf(ssw), Qe, Qo)
                return sw, ssw

            # schedule: pairs of groups; within a pair, chunk columns
            sq_idx = 0
            for pair in range(2):
                bgA, bgB = 2 * pair, 2 * pair + 1
                for c0 in range(0, _W * _C, _F):
                    swA, sswA = front(bgA, c0, sq_idx % 3 == 2)
                    sq_idx += 1
                    swB, sswB = front(bgB, c0, sq_idx % 3 == 2)
                    sq_idx += 1
                    o = ot.tile([_NP, _FH], f32, tag="o")
                    for pc in range(0, _FH, _PC):
                        s_ps = ps.tile([_NP, _PC], f32, tag="s")
                        q_ps = ps.tile([_NP, _PC], f32, tag="q")
                        nc.tensor.matmul(
                            s_ps[:], WA[:], swA[:, pc:pc + _PC],
                            start=True, stop=False,
                        )
                        nc.tensor.matmul(
                            s_ps[:], WB[:], swB[:, pc:pc + _PC],
                            start=False, stop=False,
                        )
                        nc.tensor.matmul(
                            s_ps[:], WE[:], epsr[:],
                            start=False, stop=True,
                        )
                        nc.tensor.matmul(
                            q_ps[:], WA[:], sswA[:, pc:pc + _PC],
                            start=True, stop=False,
                        )
                        nc.tensor.matmul(
                            q_ps[:], WB[:], sswB[:, pc:pc + _PC],
                            start=False, stop=True,
                        )
                        t = tmp.tile([_NP, _PC], f32, tag="t")
                        nc.vector.reciprocal_approx_fast(t[:], s_ps[:])
                        nc.vector.tensor_mul(o[:, pc:pc + _PC], q_ps[:], t[:])
                    nc.sync.dma_start(
                        yo[pair, :, c0 // 2:c0 // 2 + _FH], o[:]
                    )

    nc.compile()
    return nc


def _get_nc():
    if "nc" not in _CACHE:
        _CACHE["nc"] = _build_nc()
    return _CACHE["nc"]


def kernel(x: np.ndarray) -> np.ndarray:
    from concourse.bass_utils import run_bass_kernel_spmd

    nc = _get_nc()
    x = np.ascontiguousarray(np.asarray(x, dtype=np.float32))
    shards = np.split(x, 8, axis=0)
    in_maps = [{"x": s} for s in shards]
    res = run_bass_kernel_spmd(nc, in_maps, list(range(8)))
    return np.concatenate([res.results[i]["y"] for i in range(8)], axis=0)


# revision 15
# speedup vs baseline: 1.0635x; 1.0470x over previous
"""AcceptRejectPooling2D on 8 Trainium2 NeuronCores.

Reference semantics (per 2x2 window, stride 2, NHWC):
    r  = relu(x)
    s  = sum(r); ss = sum(r*r)
    out = ss / s   if s > 0 else 0

Sharding: pure data parallel over batch (64 -> 8 per core). Each core
processes x_local [8, 64, 64, 256] -> y_local [8, 32, 32, 256].

Pipeline (v4): partitions carry (bb, h) = 2 batches x 64 input rows.
  ACT:  R = relu(x) -> bf16
  DVE:  w-pair adds in bf16 (2x packed mode); Q = R*R on 2/3 of chunks
  PE :  h-pair reduction as matmul with a 0/1 halving matrix
        W[k, m] = 1 iff m == k//2; two groups pack into one PSUM tile
        ([0:64) from group A via W_A, [64:128) from group B via W_B);
        a third K=1 matmul accumulates eps into s so 0-windows give 0/eps
  DVE:  t = 1/s (custom recip), out = ss * t, both reading PSUM f32
"""

import sys

if "/opt/trn_rl_repo" not in sys.path:
    sys.path.insert(0, "/opt/trn_rl_repo")

import numpy as np

_B, _H, _W, _C = 8, 64, 64, 256  # per-core shard
_HO, _WO = _H // 2, _W // 2
_NP = 128                         # SBUF partitions
_F = 4096                         # floats per row chunk (16 w * 256 c)
_FH = _F // 2                     # w-pair reduced width
_PC = 512                         # psum piece width (f32, 1 bank)
_NG = 4                           # groups of (bb=2, h=64) partition rows
_EPS = 1e-30

_CACHE = {}


def _pin_act_table(bacc, mybir):
    """Route every activation to natural_log_exp_and_others so the kernel
    needs exactly one ACT function-table load. The compiler's greedy set
    choice otherwise alternates sets (~2.7us reload each)."""
    if getattr(bacc, "_arp_act_pin", False):
        return
    AF = mybir.ActivationFunctionType
    pin = {AF.Relu, AF.Square, AF.Ln, AF.Exp}
    orig = bacc.get_activation_tables

    def pinned(arch):
        tabs = orig(arch)
        keep = {f for f in pin if f in tabs.get("natural_log_exp_and_others", set())}
        return {
            name: (fns if name == "natural_log_exp_and_others" else fns - keep)
            for name, fns in tabs.items()
        }

    bacc.get_activation_tables = pinned
    bacc._arp_act_pin = True


def _build_nc():
    import concourse.bacc as bacc
    import concourse.tile as tile
    from concourse import mybir

    _pin_act_table(bacc, mybir)
    nc = bacc.Bacc("TRN2", target_bir_lowering=False, debug=False, num_devices=8)
    f32 = mybir.dt.float32
    bf16 = mybir.dt.bfloat16
    i32 = mybir.dt.int32
    x = nc.dram_tensor("x", [_B, _H, _W, _C], f32, kind="ExternalInput")
    y = nc.dram_tensor("y", [_B, _HO, _WO, _C], f32, kind="ExternalOutput")

    # [4, 128, 16384]: xg[bg, (bb, h), (w, c)]
    xg = x.ap().rearrange("(bg bb) h w c -> bg (bb h) (w c)", bb=2)
    # [2, 128, 8192]: yo[pair, (half, bb, ho), (wo, c)]
    yo = y.ap().rearrange("(pr hf bb) ho w c -> pr (hf bb ho) (w c)", pr=2, hf=2)

    relu = mybir.ActivationFunctionType.Relu
    square = mybir.ActivationFunctionType.Square
    is_ge = mybir.AluOpType.is_ge
    is_lt = mybir.AluOpType.is_lt

    with tile.TileContext(nc) as tc:
        with (
            tc.tile_pool(name="io", bufs=4) as io,
            tc.tile_pool(name="rq", bufs=8) as rq,
            tc.tile_pool(name="tmp", bufs=4) as tmp,
            tc.tile_pool(name="ot", bufs=3) as ot,
            tc.tile_pool(name="wt", bufs=1) as wt,
            tc.psum_pool(name="ps", bufs=3) as ps,
        ):
            # --- one-time: build the two halving matrices in SBUF ---
            # W_A[k, j] = 1 iff j == k//2       (cols 64.. are zero)
            # W_B[k, j] = 1 iff j == 64 + k//2  (cols ..64 are zero)
            WA = wt.tile([_NP, _NP], bf16, tag="WA")
            WB = wt.tile([_NP, _NP], bf16, tag="WB")
            d = wt.tile([_NP, _NP], i32, tag="d")
            ge = wt.tile([_NP, _NP], i32, tag="ge")
            lt = wt.tile([_NP, _NP], i32, tag="lt")
            wi = wt.tile([_NP, _NP], i32, tag="wi")
            for W, base in ((WA, 0), (WB, 128)):
                # d[p, j] = base + p - 2j; W = (d >= 0) & (d < 2)
                nc.gpsimd.iota(d[:], [[-2, _NP]], base=base, channel_multiplier=1)
                nc.vector.tensor_scalar(ge[:], d[:], 0, None, op0=is_ge)
                nc.vector.tensor_scalar(lt[:], d[:], 2, None, op0=is_lt)
                nc.vector.tensor_mul(wi[:], ge[:], lt[:])
                nc.vector.tensor_copy(W[:], wi[:])

            # warm the ACT table load + DVE recip custom-op path
            warm0 = wt.tile([_NP, 8], f32, tag="warm0")
            warm1 = wt.tile([_NP, 8], f32, tag="warm1")
            warmb = wt.tile([_NP, 8], bf16, tag="warmb")
            nc.vector.memset(warm0[:], 1.0)
            nc.scalar.activation(warmb[:], warm0[:], relu)
            nc.scalar.activation(warmb[:], warmb[:], square)
            nc.vector.reciprocal_approx_fast(warm1[:], warm0[:])

            # eps injectors: ones[1,128].T @ epsrow[1,N] accumulates eps
            # into every element of an s psum tile (runs on the idle PE)
            WE = wt.tile([1, _NP], bf16, tag="WE")
            epsr = wt.tile([1, _PC], bf16, tag="epsr")
            nc.vector.memset(WE[:], 1.0)
            nc.vector.memset(epsr[:], _EPS)

            def front(bg, c0, act_square):
                """Load + relu + square + w-pair adds for one group chunk.
                Returns (sw, ssw) bf16 [128, F/2] tiles."""
                EO = io.tile([_NP, _F], f32, tag="EO")
                nc.sync.dma_start(EO[:], xg[bg, :, c0:c0 + _F])
                R = rq.tile([_NP, _F], bf16, tag="RQ")
                Q = rq.tile([_NP, _F], bf16, tag="RQ")
                sw = tmp.tile([_NP, _FH], bf16, tag="sw")
                ssw = tmp.tile([_NP, _FH], bf16, tag="ssw")

                def wpair(t_):
                    v = t_[:].rearrange("p (w par c) -> p w par c", par=2, c=_C)
                    return v[:, :, 0, :], v[:, :, 1, :]

                def whalf(t_):
                    return t_[:].rearrange("p (w c) -> p w c", c=_C)

                nc.scalar.activation(R[:], EO[:], relu)
                Re, Ro = wpair(R)
                nc.vector.tensor_add(whalf(sw), Re, Ro)
                if act_square:
                    nc.scalar.activation(Q[:], R[:], square)
                else:
                    nc.vector.tensor_mul(Q[:], R[:], R[:])
                Qe, Qo = wpair(Q)
                nc.vector.tensor_add(whalf(ssw), Qe, Qo)
                return sw, ssw

            # schedule: pairs of groups; within a pair, chunk columns
            sq_idx = 0
            for pair in range(2):
                bgA, bgB = 2 * pair, 2 * pair + 1
                for c0 in range(0, _W * _C, _F):
                    swA, sswA = front(bgA, c0, sq_idx % 3 == 2)
                    sq_idx += 1
                    swB, sswB = front(bgB, c0, sq_idx % 3 == 2)
                    sq_idx += 1
                    o = ot.tile([_NP, _FH], f32, tag="o")
                    for pc in range(0, _FH, _PC):
                        s_ps = ps.tile([_NP, _PC], f32, tag="s")
                        q_ps = ps.tile([_NP, _PC], f32, tag="q")
                        nc.tensor.matmul(
                            s_ps[:], WA[:], swA[:, pc:pc + _PC],
                            start=True, stop=False,
                        )
                        nc.tensor.matmul(
                            s_ps[:], WB[:], swB[:, pc:pc + _PC],
                            start=False, stop=False,
                        )
                        nc.tensor.matmul(
                            s_ps[:], WE[:], epsr[:],
                            start=False, stop=True,
                        )
                        nc.tensor.matmul(
                            q_ps[:], WA[:], sswA[:, pc:pc + _PC],
                            start=True, stop=False,
                        )
                        nc.tensor.matmul(
                            q_ps[:], WB[:], sswB[:, pc:pc + _PC],
                            start=False, stop=True,
                        )
                        t = tmp.tile([_NP, _PC], f32, tag="t")
                        nc.vector.reciprocal_approx_fast(t[:], s_ps[:])
                        nc.vector.tensor_mul(o[:, pc:pc + _PC], q_ps[:], t[:])
                    # stores go out on the ACT HWDGE queue so their sem
                    # waits don't head-of-line block loads on the sync queue
                    nc.scalar.dma_start(
                        yo[pair, :, c0 // 2:c0 // 2 + _FH], o[:]
                    )

    nc.compile()
    return nc


def _get_nc():
    if "nc" not in _CACHE:
        _CACHE["nc"] = _build_nc()
    return _CACHE["nc"]


def kernel(x: np.ndarray) -> np.ndarray:
    from concourse.bass_utils import run_bass_kernel_spmd

    nc = _get_nc()
    x = np.ascontiguousarray(np.asarray(x, dtype=np.float32))
    shards = np.split(x, 8, axis=0)
    in_maps = [{"x": s} for s in shards]
    res = run_bass_kernel_spmd(nc, in_maps, list(range(8)))
    return np.concatenate([res.results[i]["y"] for i in range(8)], axis=0)
